# revision 6
# baseline (speedup 1.0000x reference)
"""Two-layer GCN (PyG GCNConv x2 + ReLU) on 8 Trainium2 NeuronCores.

Strategy (graph/data parallel, destination-partitioned):
  - Nodes row-sharded across 8 cores (6250 real + pad -> 6272 per core).
  - Edges partitioned by destination owner, grouped by destination tile
    (128 dst nodes) and by source-row region:
      region A: source local row in [0, 3200)  -> tableA rows (8*3200=25600)
      region B: source local row in [3200, 6272) -> tableB rows (8*3072=24576)
    so gather indices always fit in int16 AND the per-region AllGather can
    be overlapped with gathers of the other region.
  - Per layer on each core:  h^T = W^T @ x^T on PE;  g^T = h^T * dinv[src];
    g rows staged to DRAM in two halves, each AllGather'ed separately.
    Per-edge source rows fetched with SWDGE dma_gather (4 queues).
    Segment-sum by destination via PE matmul against a one-hot S matrix
    built ON DEVICE (iota == dstid, scaled by dinv[dst]) -- no S traffic
    from HBM.  Self-loop term g*dinv folded into the epilogue.
  - fp16 operands with fp32 PSUM accumulation.
"""

import math
import os
import sys

import numpy as np

for _p in ("/opt/trn_rl_repo", "/root/.axon_site/_ro/trn_rl_repo"):
    if os.path.isdir(_p) and _p not in sys.path:
        sys.path.append(_p)

import concourse.bacc as bacc
import concourse.bass as bass
import concourse.mybir as mybir
import concourse.tile as tile
from concourse.bass_utils import run_bass_kernel_spmd

# Problem constants (hardcoded per harness contract).
N, E, IN, HID, OUT = 50000, 800000, 128, 128, 64
NCORES = 8
NPC_REAL = N // NCORES          # 6250
TILES = 49
NPC = TILES * 128               # 6272 padded nodes per core
SPLIT = 3200                    # region A local rows (25 tiles)
NB = NPC - SPLIT                # 3072 region B local rows (24 tiles)
TA = SPLIT // 128               # 25 tiles in region A half
TB = NB // 128                  # 24
RA = NCORES * SPLIT             # 25600 tableA rows
RB = NCORES * NB                # 24576 tableB rows
WV = 16                         # gather wave size in chunks (128 slots each)

F16 = mybir.dt.float16
F32 = mybir.dt.float32
NPF16 = np.float16

_ts = bass.ts


def default_cfg():
    return dict(N=N, E=E, IN=IN, HID=HID, OUT=OUT, NCORES=NCORES,
                NPC_REAL=NPC_REAL, TILES=TILES, NPC=NPC, WV=WV)


def _preprocess(edge_index: np.ndarray, cfg=None):
    """Partition/sort/pad edges; build per-core gather-index, dst-id and
    dst-norm arrays plus the shared static chunk schedule."""
    src = np.asarray(edge_index[0], np.int64)
    dst = np.asarray(edge_index[1], np.int64)
    deg = np.bincount(dst, minlength=N).astype(np.float64) + 1.0
    dinv = 1.0 / np.sqrt(deg)

    owner = dst // NPC_REAL
    dst_local = dst % NPC_REAL
    so = src // NPC_REAL
    sl = src % NPC_REAL
    region = (sl >= SPLIT).astype(np.int64)
    row = np.where(region == 0, so * SPLIT + sl, so * NB + (sl - SPLIT))
    tile_id = dst_local // 128
    intile = dst_local % 128

    cnt = np.zeros((NCORES, TILES, 2), np.int64)
    np.add.at(cnt, (owner, tile_id, region), 1)
    K = np.ceil(cnt.max(axis=0) / 128).astype(np.int64)   # [TILES, 2]
    K_A, K_B = K[:, 0].copy(), K[:, 1].copy()
    C_A, C_B = int(K_A.sum()), int(K_B.sum())
    C = C_A + C_B
    LB = np.concatenate([[0], np.cumsum(K_A)[:-1]]).astype(np.int64)
    HB = (C_A + np.concatenate([[0], np.cumsum(K_B)[:-1]])).astype(np.int64)

    # slot assignment: sort edges by (owner, tile, region); position within
    # each (owner, tile, region) group via cumulative count
    gid = (owner * TILES + tile_id) * 2 + region
    order = np.lexsort((row, gid))
    gs = gid[order]
    starts = np.concatenate([[0], np.flatnonzero(np.diff(gs)) + 1])
    group_of = np.searchsorted(starts, np.arange(len(gs)), side="right") - 1
    pos = np.arange(len(gs)) - starts[group_of]

    base_chunk = np.where(region == 0, LB[tile_id], HB[tile_id])
    slot = np.empty(len(gs), np.int64)
    slot[order] = base_chunk[order] * 128 + pos

    nslots = C * 128
    idx16 = np.zeros((NCORES, nslots), np.int16)
    dstid = np.full((NCORES, nslots), -1.0, np.float32)
    idx16[owner, slot] = row.astype(np.int16)
    dstid[owner, slot] = intile

    # wrapped+replicated gather index tile [128, C*8]
    idx_t = idx16.reshape(NCORES, C * 8, 16).transpose(0, 2, 1)     # [8,16,C*8]
    idx_t = np.tile(idx_t, (1, 8, 1)).copy()                        # [8,128,C*8]
    dstid_t = dstid.reshape(NCORES, C, 128).transpose(0, 2, 1)      # [8,128,C]
    dstid_t = dstid_t.astype(np.float32).copy()

    return dict(deg=deg, dinv=dinv, K_A=K_A, K_B=K_B, C_A=C_A, C_B=C_B,
                C=C, LB=LB, HB=HB, idx_t=idx_t, dstid_t=dstid_t)


def _waves(n_chunks: int, wv: int = WV):
    out = []
    c = 0
    while c < n_chunks:
        n = min(wv, n_chunks - c)
        out.append((c, n))
        c += n
    return out


def _build_program(meta, cfg=None):
    stage = (cfg or {}).get("stage", os.environ.get("GCN_STAGE", "full"))
    K_A, K_B = meta["K_A"], meta["K_B"]
    C_A, C_B, C = meta["C_A"], meta["C_B"], meta["C"]
    LB, HB = meta["LB"], meta["HB"]

    nc = bacc.Bacc("TRN2", target_bir_lowering=False, debug=False,
                   num_devices=NCORES, num_swdge_queues=4)

    # ---- I/O ----
    xT_d = nc.dram_tensor("xT", [IN, NPC], F16, kind="ExternalInput")
    w1_d = nc.dram_tensor("W1", [IN, HID], F16, kind="ExternalInput")
    w2_d = nc.dram_tensor("W2", [HID, OUT], F16, kind="ExternalInput")
    b1_d = nc.dram_tensor("b1c", [HID, 1], F32, kind="ExternalInput")
    b2_d = nc.dram_tensor("b2c", [OUT, 1], F32, kind="ExternalInput")
    dinv_d = nc.dram_tensor("dinvrep", [128, NPC], F16, kind="ExternalInput")
    ident_d = nc.dram_tensor("ident", [128, 128], F16, kind="ExternalInput")
    idx_d = nc.dram_tensor("idxt", [128, C * 8], mybir.dt.int16,
                           kind="ExternalInput")
    dstid_d = nc.dram_tensor("dstid", [128, C], F32, kind="ExternalInput")
    iota_d = nc.dram_tensor("iota", [128, 128], F16, kind="ExternalInput")
    out_d = nc.dram_tensor("outT", [OUT, NPC], F32, kind="ExternalOutput")

    # ---- internal DRAM (collective bounce + replicated tables) ----
    gdA = [nc.dram_tensor(f"gdA{l}", [SPLIT, 128], F16) for l in (1, 2)]
    gdB = [nc.dram_tensor(f"gdB{l}", [NB, 128], F16) for l in (1, 2)]
    tabA = [nc.dram_tensor(f"tabA{l}", [RA, 128], F16, addr_space="Shared")
            for l in (1, 2)]
    tabB = [nc.dram_tensor(f"tabB{l}", [RB, 128], F16, addr_space="Shared")
            for l in (1, 2)]

    rg = [list(range(NCORES))]
    a_waves = _waves(C_A)
    b_waves = _waves(C_B)

    with tile.TileContext(nc) as tc:
        with (
            tc.tile_pool(name="const", bufs=1) as constp,
            tc.tile_pool(name="xtp", bufs=1) as xtp,
            tc.tile_pool(name="gtp", bufs=2) as gtp,
            tc.tile_pool(name="stp", bufs=2) as stp,
            tc.tile_pool(name="rlp", bufs=1) as rlp,
            tc.tile_pool(name="prp", bufs=2) as prp,
            tc.tile_pool(name="wap", bufs=6) as wap,
            tc.tile_pool(name="wbp", bufs=6) as wbp,
            tc.tile_pool(name="sp", bufs=8) as sp,
            tc.tile_pool(name="outp", bufs=3) as outp,
            tc.tile_pool(name="tmpp", bufs=4) as tmpp,
            tc.tile_pool(name="pmm", bufs=2, space="PSUM") as pmm,
            tc.tile_pool(name="ptr", bufs=2, space="PSUM") as ptr,
            tc.tile_pool(name="psc", bufs=4, space="PSUM") as psc,
        ):
            # ---- constants / inputs to SBUF ----
            xT = xtp.tile([128, NPC], F16, tag="xT")
            nc.sync.dma_start(xT[:IN, :], xT_d[:, :])
            dinv = constp.tile([128, NPC], F16, tag="dinv")
            nc.scalar.dma_start(dinv[:], dinv_d[:, :])
            idxt = constp.tile([128, C * 8], mybir.dt.int16, tag="idxt")
            nc.sync.dma_start(idxt[:], idx_d[:, :])
            w1 = constp.tile([IN, HID], F16, tag="w1")
            nc.scalar.dma_start(w1[:], w1_d[:, :])
            w2 = constp.tile([HID, OUT], F16, tag="w2")
            nc.scalar.dma_start(w2[:], w2_d[:, :])
            b1 = constp.tile([HID, 1], F32, tag="b1")
            nc.scalar.dma_start(b1[:], b1_d[:, :])
            b2 = constp.tile([OUT, 1], F32, tag="b2")
            nc.scalar.dma_start(b2[:], b2_d[:, :])
            ident = constp.tile([128, 128], F16, tag="ident")
            nc.scalar.dma_start(ident[:], ident_d[:, :])
            dstid = constp.tile([128, C], F32, tag="dstid")
            nc.scalar.dma_start(dstid[:], dstid_d[:, :])
            iota = constp.tile([128, 128], F16, tag="iota")
            nc.scalar.dma_start(iota[:], iota_d[:, :])

            state = dict(qn=0, s_eng=0)

            def dense_half(rhs, W, Fd, gT, c0, c1):
                """gT[:, c0:c1] = (W^T @ rhs[:, c0:c1]) * dinv  (fp16)."""
                i = c0
                while i < c1:
                    w_ = min(512, c1 - i)
                    sl = slice(i, i + w_)
                    ps = pmm.tile([128, 512], F32, tag="pmm")
                    nc.tensor.matmul(ps[:Fd, :w_], W[:, :Fd], rhs[:128, sl],
                                     start=True, stop=True)
                    nc.vector.scalar_tensor_tensor(
                        gT[:Fd, sl], ps[:Fd, :w_], 0.0, dinv[:Fd, sl],
                        mybir.AluOpType.bypass, mybir.AluOpType.mult)
                    i += w_

            def stage_half(gT, Fd, layer, half):
                """Transpose per-tile into row-major, DMA to DRAM, AllGather."""
                t0, nt = (0, TA) if half == 0 else (TA, TB)
                ncols = nt * 128
                staged = stp.tile([128, TA * 128], F16, tag="staged")
                if Fd < 128:
                    nc.vector.memset(staged[:, :ncols], 0.0)
                for t in range(t0, t0 + nt):
                    pt = ptr.tile([128, 128], F16, tag="ptr")
                    nc.tensor.transpose(pt[:, :Fd], gT[:Fd, _ts(t, 128)],
                                        ident[:Fd, :Fd])
                    dstc = (t - t0) * 128
                    nc.scalar.copy(staged[:, dstc:dstc + Fd], pt[:, :Fd])
                gd = gdA[layer] if half == 0 else gdB[layer]
                tab = tabA[layer] if half == 0 else tabB[layer]
                gview = gd.ap().rearrange("(t p) f -> p t f", p=128)
                sview = staged[:, :ncols].rearrange("p (t f) -> p t f", f=128)
                nc.sync.dma_start(gview, sview)
                nc.gpsimd.collective_compute(
                    "AllGather", mybir.AluOpType.bypass, replica_groups=rg,
                    ins=[gd.ap()], outs=[tab.ap()])

            def make_wave_cache(layer):
                cache = {}

                def ensure_wave(rgn, wi):
                    key = (rgn, wi)
                    if key in cache:
                        return cache[key]
                    c0, n = (a_waves if rgn == 0 else b_waves)[wi]
                    if rgn == 1:
                        c0 += C_A
                    pool = wap if rgn == 0 else wbp
                    g = pool.tile([128, WV, 128], F16,
                                  tag="wa" if rgn == 0 else "wb")
                    tab = tabA[layer] if rgn == 0 else tabB[layer]
                    nr = RA if rgn == 0 else RB
                    qn = state["qn"]
                    state["qn"] = (qn + 1) % 4
                    nc.gpsimd.dma_gather(
                        g[:, :n, :], tab[0:nr, :],
                        idxt[:, c0 * 8:(c0 + n) * 8],
                        n * 128, n * 128, 128, single_packet=False,
                        queue_num=qn)
                    cache[key] = g
                    return g
                return ensure_wave

            def ensure_s(ch, s_cache):
                st = s_cache.get(ch)
                if st is not None:
                    return st
                st = sp.tile([128, 128], F16, tag="s")
                eng = nc.gpsimd if state["s_eng"] == 2 else nc.vector
                state["s_eng"] = (state["s_eng"] + 1) % 3
                eng.tensor_scalar(
                    st[:, :], iota[:, :], dstid[:, ch:ch + 1],
                    scalar2=None,
                    op0=mybir.AluOpType.is_equal)
                s_cache[ch] = st
                return st

            def pass_region(rgn, Fd, t_range, ensure_wave, s_cache, finish):
                Kr = K_A if rgn == 0 else K_B
                Base = LB if rgn == 0 else HB
                off = 0 if rgn == 0 else C_A
                for t in t_range:
                    k = int(Kr[t])
                    if k == 0:
                        finish(t, None)
                        continue
                    pscat = psc.tile([128, 128], F32, tag="psc")
                    for j in range(k):
                        ch = int(Base[t]) + j
                        rel = ch - off
                        g = ensure_wave(rgn, rel // WV)
                        pos = rel % WV
                        st = ensure_s(ch, s_cache)
                        nc.tensor.matmul(
                            pscat[:Fd, :], g[:, pos, :Fd], st[:, :],
                            start=(j == 0), stop=(j == k - 1))
                    finish(t, pscat)

            relu1 = rlp.tile([128, NPC], F16, tag="relu1")

            if stage == "ag":
                gT = gtp.tile([128, NPC], F16, tag="gT")
                dense_half(xT, w1, HID, gT, 0, SPLIT)
                stage_half(gT, HID, 0, 0)
                dense_half(xT, w1, HID, gT, SPLIT, NPC)
                stage_half(gT, HID, 0, 1)
                nc.vector.memset(relu1[:, :], 0.0)
                gT2 = gtp.tile([128, NPC], F16, tag="gT")
                dense_half(relu1, w2, OUT, gT2, 0, SPLIT)
                stage_half(gT2, OUT, 1, 0)
                dense_half(relu1, w2, OUT, gT2, SPLIT, NPC)
                stage_half(gT2, OUT, 1, 1)
                for t in range(TILES):
                    ot = outp.tile([OUT, 128], F32, tag="out")
                    nc.vector.memset(ot[:], 0.0)
                    nc.sync.dma_start(out_d[:, _ts(t, 128)], ot[:])

            gT_next = None
            for layer in ((0, 1) if stage != "ag" else ()):
                Fd = HID if layer == 0 else OUT
                bcol = b1 if layer == 0 else b2

                if layer == 0:
                    gT = gtp.tile([128, NPC], F16, tag="gT")
                    dense_half(xT, w1, Fd, gT, 0, SPLIT)
                    stage_half(gT, Fd, 0, 0)
                    dense_half(xT, w1, Fd, gT, SPLIT, NPC)
                    stage_half(gT, Fd, 0, 1)
                else:
                    gT = gT_next  # computed during layer-0 pass B

                ensure_wave = make_wave_cache(layer)
                s_cache = {}

                if stage == "gather":
                    # issue every gather, then write dummy outputs
                    for wi in range(len(a_waves)):
                        ensure_wave(0, wi)
                    for wi in range(len(b_waves)):
                        ensure_wave(1, wi)
                    if layer == 0:
                        nc.vector.memset(relu1[:, :], 0.0)
                        gT_next = gtp.tile([128, NPC], F16, tag="gT")
                        dense_half(relu1, w2, OUT, gT_next, 0, SPLIT)
                        stage_half(gT_next, OUT, 1, 0)
                        dense_half(relu1, w2, OUT, gT_next, SPLIT, NPC)
                        stage_half(gT_next, OUT, 1, 1)
                    else:
                        for t in range(TILES):
                            ot = outp.tile([OUT, 128], F32, tag="out")
                            nc.vector.memset(ot[:], 0.0)
                            nc.sync.dma_start(out_d[:, _ts(t, 128)], ot[:])
                    continue

                # ---- pass A: region-A chunks -> partial (+ self term) ----
                partial = prp.tile([128, NPC], F16, tag="partial")

                def finA(t, pscat, _p=partial, _g=gT, _f=Fd):
                    sl = _ts(t, 128)
                    if pscat is None:
                        nc.vector.tensor_copy(_p[:_f, sl], _g[:_f, sl])
                    else:
                        nc.vector.tensor_add(_p[:_f, sl], pscat[:_f, :],
                                             _g[:_f, sl])

                pass_region(0, Fd, range(TILES), ensure_wave, s_cache, finA)

                # ---- pass B: region-B chunks -> epilogue ----
                def finB(t, pscat, _p=partial, _f=Fd, _b=bcol, _l=layer):
                    sl = _ts(t, 128)
                    tmp2 = tmpp.tile([128, 128], F32, tag="tmp")
                    if pscat is None:
                        nc.vector.scalar_tensor_tensor(
                            tmp2[:_f, :], _p[:_f, sl], 0.0, dinv[:_f, sl],
                            mybir.AluOpType.bypass, mybir.AluOpType.mult)
                    else:
                        tmp = tmpp.tile([128, 128], F32, tag="tmp")
                        nc.vector.tensor_add(tmp[:_f, :], pscat[:_f, :],
                                             _p[:_f, sl])
                        nc.vector.scalar_tensor_tensor(
                            tmp2[:_f, :], tmp[:_f, :], 0.0, dinv[:_f, sl],
                            mybir.AluOpType.bypass, mybir.AluOpType.mult)
                    src_ap = tmp2[:_f, :]
                    if _l == 0:
                        nc.scalar.activation(
                            relu1[:_f, sl], src_ap,
                            mybir.ActivationFunctionType.Relu,
                            bias=_b[:_f, :], scale=1.0)
                    else:
                        ot = outp.tile([OUT, 128], F32, tag="out")
                        nc.scalar.activation(
                            ot[:], src_ap,
                            mybir.ActivationFunctionType.Relu,
                            bias=_b[:_f, :], scale=1.0)
                        nc.sync.dma_start(out_d[:, sl], ot[:])

                if layer == 0:
                    # tiles 0..TA-1 -> relu1 cols [0, SPLIT) -> start layer-1
                    # dense+AG for half A while pass B continues
                    pass_region(1, Fd, range(TA), ensure_wave, s_cache, finB)
                    gT_next = gtp.tile([128, NPC], F16, tag="gT")
                    dense_half(relu1, w2, OUT, gT_next, 0, SPLIT)
                    stage_half(gT_next, OUT, 1, 0)
                    pass_region(1, Fd, range(TA, TILES), ensure_wave,
                                s_cache, finB)
                    dense_half(relu1, w2, OUT, gT_next, SPLIT, NPC)
                    stage_half(gT_next, OUT, 1, 1)
                else:
                    pass_region(1, Fd, range(TILES), ensure_wave,
                                s_cache, finB)

    nc.compile()
    return nc


def _host_inputs(inputs, meta, cfg=None):
    x = np.asarray(inputs["x"], np.float32)
    W1 = np.asarray(inputs["W1"], np.float32)
    b1 = np.asarray(inputs["b1"], np.float32)
    W2 = np.asarray(inputs["W2"], np.float32)
    b2 = np.asarray(inputs["b2"], np.float32)
    dinv = meta["dinv"]

    ident = np.eye(128, dtype=NPF16)
    w1c = W1.astype(NPF16)
    w2c = W2.astype(NPF16)
    b1c = b1.reshape(HID, 1).astype(np.float32)
    b2c = b2.reshape(OUT, 1).astype(np.float32)
    iota = np.tile(np.arange(128, dtype=NPF16)[None, :], (128, 1)).copy()

    in_maps = []
    for c in range(NCORES):
        xs = np.zeros((NPC, IN), np.float32)
        xs[:NPC_REAL] = x[c * NPC_REAL:(c + 1) * NPC_REAL]
        xT = np.ascontiguousarray(xs.T).astype(NPF16)

        dv = np.ones(NPC, np.float32)
        dv[:NPC_REAL] = dinv[c * NPC_REAL:(c + 1) * NPC_REAL]
        dinvrep = np.tile(dv[None, :], (128, 1)).astype(NPF16)

        in_maps.append({
            "xT": xT, "W1": w1c, "W2": w2c, "b1c": b1c, "b2c": b2c,
            "dinvrep": dinvrep, "ident": ident,
            "idxt": meta["idx_t"][c], "dstid": meta["dstid_t"][c],
            "iota": iota,
        })
    return in_maps


def kernel(**inputs) -> np.ndarray:
    meta = _preprocess(np.asarray(inputs["edge_index"]))
    nc = _build_program(meta)
    in_maps = _host_inputs(inputs, meta)
    res = run_bass_kernel_spmd(nc, in_maps, list(range(NCORES)))
    out = np.empty((N, OUT), np.float32)
    for c in range(NCORES):
        out[c * NPC_REAL:(c + 1) * NPC_REAL] = \
            res.results[c]["outT"][:, :NPC_REAL].T
    return out


# revision 7
# speedup vs baseline: 1.8029x; 1.8029x over previous
"""Two-layer GCN (PyG GCNConv x2 + ReLU) on 8 Trainium2 NeuronCores.

Strategy (graph/data parallel, destination-partitioned):
  - Nodes row-sharded across 8 cores (6250 real + pad -> 6272 per core).
  - Edges partitioned by destination owner, grouped by destination tile
    (128 dst nodes) and by source-row region:
      region A: source local row in [0, 3200)  -> tableA rows (8*3200=25600)
      region B: source local row in [3200, 6272) -> tableB rows (8*3072=24576)
    so gather indices always fit in int16 AND the per-region AllGather can
    be overlapped with gathers of the other region.
  - Per layer on each core:  h^T = W^T @ x^T on PE;  g^T = h^T * dinv[src];
    g rows staged to DRAM in two halves, each AllGather'ed separately.
    Per-edge source rows fetched with SWDGE dma_gather (4 queues).
    Segment-sum by destination via PE matmul against a one-hot S matrix
    built ON DEVICE (iota == dstid, scaled by dinv[dst]) -- no S traffic
    from HBM.  Self-loop term g*dinv folded into the epilogue.
  - fp16 operands with fp32 PSUM accumulation.
"""

import math
import os
import sys

import numpy as np

for _p in ("/opt/trn_rl_repo", "/root/.axon_site/_ro/trn_rl_repo"):
    if os.path.isdir(_p) and _p not in sys.path:
        sys.path.append(_p)

import concourse.bacc as bacc
import concourse.bass as bass
import concourse.mybir as mybir
import concourse.tile as tile
from concourse.bass_utils import run_bass_kernel_spmd

# Problem constants (hardcoded per harness contract).
N, E, IN, HID, OUT = 50000, 800000, 128, 128, 64
NCORES = 8
NPC_REAL = N // NCORES          # 6250
TILES = 49
NPC = TILES * 128               # 6272 padded nodes per core
SPLIT = 3200                    # region A local rows (25 tiles)
NB = NPC - SPLIT                # 3072 region B local rows (24 tiles)
TA = SPLIT // 128               # 25 tiles in region A half
TB = NB // 128                  # 24
RA = NCORES * SPLIT             # 25600 tableA rows
RB = NCORES * NB                # 24576 tableB rows
WV = 16                         # gather wave size in chunks (128 slots each)

F16 = mybir.dt.float16
F32 = mybir.dt.float32
NPF16 = np.float16

_ts = bass.ts


def default_cfg():
    return dict(N=N, E=E, IN=IN, HID=HID, OUT=OUT, NCORES=NCORES,
                NPC_REAL=NPC_REAL, TILES=TILES, NPC=NPC, WV=WV)


def _preprocess(edge_index: np.ndarray, cfg=None):
    """Partition/sort/pad edges; build per-core gather-index, dst-id and
    dst-norm arrays plus the shared static chunk schedule."""
    src = np.asarray(edge_index[0], np.int64)
    dst = np.asarray(edge_index[1], np.int64)
    deg = np.bincount(dst, minlength=N).astype(np.float64) + 1.0
    dinv = 1.0 / np.sqrt(deg)

    owner = dst // NPC_REAL
    dst_local = dst % NPC_REAL
    so = src // NPC_REAL
    sl = src % NPC_REAL
    region = (sl >= SPLIT).astype(np.int64)
    row = np.where(region == 0, so * SPLIT + sl, so * NB + (sl - SPLIT))
    tile_id = dst_local // 128
    intile = dst_local % 128

    cnt = np.zeros((NCORES, TILES, 2), np.int64)
    np.add.at(cnt, (owner, tile_id, region), 1)
    K = np.ceil(cnt.max(axis=0) / 128).astype(np.int64)   # [TILES, 2]
    K_A, K_B = K[:, 0].copy(), K[:, 1].copy()
    C_A, C_B = int(K_A.sum()), int(K_B.sum())
    C = C_A + C_B
    LB = np.concatenate([[0], np.cumsum(K_A)[:-1]]).astype(np.int64)
    HB = (C_A + np.concatenate([[0], np.cumsum(K_B)[:-1]])).astype(np.int64)

    # slot assignment: sort edges by (owner, tile, region); position within
    # each (owner, tile, region) group via cumulative count
    gid = (owner * TILES + tile_id) * 2 + region
    order = np.lexsort((row, gid))
    gs = gid[order]
    starts = np.concatenate([[0], np.flatnonzero(np.diff(gs)) + 1])
    group_of = np.searchsorted(starts, np.arange(len(gs)), side="right") - 1
    pos = np.arange(len(gs)) - starts[group_of]

    base_chunk = np.where(region == 0, LB[tile_id], HB[tile_id])
    slot = np.empty(len(gs), np.int64)
    slot[order] = base_chunk[order] * 128 + pos

    nslots = C * 128
    idx16 = np.zeros((NCORES, nslots), np.int16)
    dstid = np.full((NCORES, nslots), -1.0, np.float32)
    idx16[owner, slot] = row.astype(np.int16)
    dstid[owner, slot] = intile

    # wrapped+replicated gather index tile [128, C*8]
    idx_t = idx16.reshape(NCORES, C * 8, 16).transpose(0, 2, 1)     # [8,16,C*8]
    idx_t = np.tile(idx_t, (1, 8, 1)).copy()                        # [8,128,C*8]
    dstid_t = dstid.reshape(NCORES, C, 128).transpose(0, 2, 1)      # [8,128,C]
    dstid_t = dstid_t.astype(np.float32).copy()

    return dict(deg=deg, dinv=dinv, K_A=K_A, K_B=K_B, C_A=C_A, C_B=C_B,
                C=C, LB=LB, HB=HB, idx_t=idx_t, dstid_t=dstid_t)


def _waves(n_chunks: int, wv: int = WV):
    out = []
    c = 0
    while c < n_chunks:
        n = min(wv, n_chunks - c)
        out.append((c, n))
        c += n
    return out


def _build_program(meta, cfg=None):
    stage = (cfg or {}).get("stage", os.environ.get("GCN_STAGE", "full"))
    K_A, K_B = meta["K_A"], meta["K_B"]
    C_A, C_B, C = meta["C_A"], meta["C_B"], meta["C"]
    LB, HB = meta["LB"], meta["HB"]

    nc = bacc.Bacc("TRN2", target_bir_lowering=False, debug=False,
                   num_devices=NCORES, num_swdge_queues=4)

    # ---- I/O ----
    xT_d = nc.dram_tensor("xT", [IN, NPC], F16, kind="ExternalInput")
    w1_d = nc.dram_tensor("W1", [IN, HID], F16, kind="ExternalInput")
    w2_d = nc.dram_tensor("W2", [HID, OUT], F16, kind="ExternalInput")
    b1_d = nc.dram_tensor("b1c", [HID, 1], F32, kind="ExternalInput")
    b2_d = nc.dram_tensor("b2c", [OUT, 1], F32, kind="ExternalInput")
    dinv_d = nc.dram_tensor("dinvrep", [128, NPC], F16, kind="ExternalInput")
    ident_d = nc.dram_tensor("ident", [128, 128], F16, kind="ExternalInput")
    idx_d = nc.dram_tensor("idxt", [128, C * 8], mybir.dt.int16,
                           kind="ExternalInput")
    dstid_d = nc.dram_tensor("dstid", [128, C], F32, kind="ExternalInput")
    iota_d = nc.dram_tensor("iota", [128, 128], F16, kind="ExternalInput")
    out_d = nc.dram_tensor("outT", [OUT, NPC], F32, kind="ExternalOutput")

    # ---- internal DRAM (collective bounce + replicated tables) ----
    gdA = [nc.dram_tensor(f"gdA{l}", [SPLIT, 128], F16) for l in (1, 2)]
    gdB = [nc.dram_tensor(f"gdB{l}", [NB, 128], F16) for l in (1, 2)]
    tabA = [nc.dram_tensor(f"tabA{l}", [RA, 128], F16, addr_space="Shared")
            for l in (1, 2)]
    tabB = [nc.dram_tensor(f"tabB{l}", [RB, 128], F16, addr_space="Shared")
            for l in (1, 2)]

    rg = [list(range(NCORES))]
    a_waves = _waves(C_A)
    b_waves = _waves(C_B)

    with tile.TileContext(nc) as tc:
        with (
            tc.tile_pool(name="const", bufs=1) as constp,
            tc.tile_pool(name="xtp", bufs=1) as xtp,
            tc.tile_pool(name="gtp", bufs=2) as gtp,
            tc.tile_pool(name="stp", bufs=2) as stp,
            tc.tile_pool(name="rlp", bufs=1) as rlp,
            tc.tile_pool(name="prp", bufs=2) as prp,
            tc.tile_pool(name="wap", bufs=6) as wap,
            tc.tile_pool(name="wbp", bufs=6) as wbp,
            tc.tile_pool(name="sp", bufs=24) as sp,
            tc.tile_pool(name="outp", bufs=3) as outp,
            tc.tile_pool(name="tmpp", bufs=4) as tmpp,
            tc.tile_pool(name="pmm", bufs=2, space="PSUM") as pmm,
            tc.tile_pool(name="ptr", bufs=2, space="PSUM") as ptr,
            tc.tile_pool(name="psc", bufs=4, space="PSUM") as psc,
        ):
            # ---- constants / inputs to SBUF ----
            xT = xtp.tile([128, NPC], F16, tag="xT")
            nc.sync.dma_start(xT[:IN, :], xT_d[:, :])
            dinv = constp.tile([128, NPC], F16, tag="dinv")
            nc.scalar.dma_start(dinv[:], dinv_d[:, :])
            idxt = constp.tile([128, C * 8], mybir.dt.int16, tag="idxt")
            nc.sync.dma_start(idxt[:], idx_d[:, :])
            w1 = constp.tile([IN, HID], F16, tag="w1")
            nc.scalar.dma_start(w1[:], w1_d[:, :])
            w2 = constp.tile([HID, OUT], F16, tag="w2")
            nc.scalar.dma_start(w2[:], w2_d[:, :])
            b1 = constp.tile([HID, 1], F32, tag="b1")
            nc.scalar.dma_start(b1[:], b1_d[:, :])
            b2 = constp.tile([OUT, 1], F32, tag="b2")
            nc.scalar.dma_start(b2[:], b2_d[:, :])
            ident = constp.tile([128, 128], F16, tag="ident")
            nc.scalar.dma_start(ident[:], ident_d[:, :])
            dstid = constp.tile([128, C], F32, tag="dstid")
            nc.scalar.dma_start(dstid[:], dstid_d[:, :])
            iota = constp.tile([128, 128], F16, tag="iota")
            nc.scalar.dma_start(iota[:], iota_d[:, :])

            state = dict(qn=0, s_eng=0)

            def dense_half(rhs, W, Fd, gT, c0, c1):
                """gT[:, c0:c1] = (W^T @ rhs[:, c0:c1]) * dinv  (fp16)."""
                i = c0
                while i < c1:
                    w_ = min(512, c1 - i)
                    sl = slice(i, i + w_)
                    ps = pmm.tile([128, 512], F32, tag="pmm")
                    nc.tensor.matmul(ps[:Fd, :w_], W[:, :Fd], rhs[:128, sl],
                                     start=True, stop=True)
                    nc.vector.scalar_tensor_tensor(
                        gT[:Fd, sl], ps[:Fd, :w_], 0.0, dinv[:Fd, sl],
                        mybir.AluOpType.bypass, mybir.AluOpType.mult)
                    i += w_

            def stage_half(gT, Fd, layer, half):
                """Transpose per-tile into row-major, DMA to DRAM, AllGather."""
                t0, nt = (0, TA) if half == 0 else (TA, TB)
                ncols = nt * 128
                staged = stp.tile([128, TA * 128], F16, tag="staged")
                if Fd < 128:
                    nc.vector.memset(staged[:, :ncols], 0.0)
                for t in range(t0, t0 + nt):
                    pt = ptr.tile([128, 128], F16, tag="ptr")
                    nc.tensor.transpose(pt[:, :Fd], gT[:Fd, _ts(t, 128)],
                                        ident[:Fd, :Fd])
                    dstc = (t - t0) * 128
                    nc.scalar.copy(staged[:, dstc:dstc + Fd], pt[:, :Fd])
                gd = gdA[layer] if half == 0 else gdB[layer]
                tab = tabA[layer] if half == 0 else tabB[layer]
                gview = gd.ap().rearrange("(t p) f -> p t f", p=128)
                sview = staged[:, :ncols].rearrange("p (t f) -> p t f", f=128)
                nc.sync.dma_start(gview, sview)
                nc.gpsimd.collective_compute(
                    "AllGather", mybir.AluOpType.bypass, replica_groups=rg,
                    ins=[gd.ap()], outs=[tab.ap()])

            def make_wave_cache(layer):
                cache = {}

                def ensure_wave(rgn, wi):
                    key = (rgn, wi)
                    if key in cache:
                        return cache[key]
                    c0, n = (a_waves if rgn == 0 else b_waves)[wi]
                    if rgn == 1:
                        c0 += C_A
                    pool = wap if rgn == 0 else wbp
                    g = pool.tile([128, WV, 128], F16,
                                  tag="wa" if rgn == 0 else "wb")
                    tab = tabA[layer] if rgn == 0 else tabB[layer]
                    nr = RA if rgn == 0 else RB
                    qn = state["qn"]
                    state["qn"] = (qn + 1) % 4
                    nc.gpsimd.dma_gather(
                        g[:, :n, :], tab[0:nr, :],
                        idxt[:, c0 * 8:(c0 + n) * 8],
                        n * 128, n * 128, 128, single_packet=False,
                        queue_num=qn)
                    cache[key] = g
                    return g
                return ensure_wave

            def ensure_s(ch, s_cache):
                st = s_cache.get(ch)
                if st is not None:
                    return st
                st = sp.tile([128, 128], F16, tag="s")
                nc.vector.tensor_scalar(
                    st[:, :], iota[:, :], dstid[:, ch:ch + 1],
                    scalar2=None,
                    op0=mybir.AluOpType.is_equal)
                s_cache[ch] = st
                return st

            def pass_region(rgn, Fd, t_range, ensure_wave, s_cache, finish):
                Kr = K_A if rgn == 0 else K_B
                Base = LB if rgn == 0 else HB
                off = 0 if rgn == 0 else C_A
                for t in t_range:
                    k = int(Kr[t])
                    if k == 0:
                        finish(t, None)
                        continue
                    pscat = psc.tile([128, 128], F32, tag="psc")
                    for j in range(k):
                        ch = int(Base[t]) + j
                        rel = ch - off
                        g = ensure_wave(rgn, rel // WV)
                        pos = rel % WV
                        st = ensure_s(ch, s_cache)
                        nc.tensor.matmul(
                            pscat[:Fd, :], g[:, pos, :Fd], st[:, :],
                            start=(j == 0), stop=(j == k - 1))
                    finish(t, pscat)

            relu1 = rlp.tile([128, NPC], F16, tag="relu1")

            if stage == "ag":
                gT = gtp.tile([128, NPC], F16, tag="gT")
                dense_half(xT, w1, HID, gT, 0, SPLIT)
                stage_half(gT, HID, 0, 0)
                dense_half(xT, w1, HID, gT, SPLIT, NPC)
                stage_half(gT, HID, 0, 1)
                nc.vector.memset(relu1[:, :], 0.0)
                gT2 = gtp.tile([128, NPC], F16, tag="gT")
                dense_half(relu1, w2, OUT, gT2, 0, SPLIT)
                stage_half(gT2, OUT, 1, 0)
                dense_half(relu1, w2, OUT, gT2, SPLIT, NPC)
                stage_half(gT2, OUT, 1, 1)
                for t in range(TILES):
                    ot = outp.tile([OUT, 128], F32, tag="out")
                    nc.vector.memset(ot[:], 0.0)
                    nc.sync.dma_start(out_d[:, _ts(t, 128)], ot[:])

            gT_next = None
            for layer in ((0, 1) if stage != "ag" else ()):
                Fd = HID if layer == 0 else OUT
                bcol = b1 if layer == 0 else b2

                if layer == 0:
                    gT = gtp.tile([128, NPC], F16, tag="gT")
                    dense_half(xT, w1, Fd, gT, 0, SPLIT)
                    stage_half(gT, Fd, 0, 0)
                    dense_half(xT, w1, Fd, gT, SPLIT, NPC)
                    stage_half(gT, Fd, 0, 1)
                else:
                    gT = gT_next  # computed during layer-0 pass B

                ensure_wave = make_wave_cache(layer)
                s_cache = {}

                if stage == "gather":
                    # issue every gather, then write dummy outputs
                    for wi in range(len(a_waves)):
                        ensure_wave(0, wi)
                    for wi in range(len(b_waves)):
                        ensure_wave(1, wi)
                    if layer == 0:
                        nc.vector.memset(relu1[:, :], 0.0)
                        gT_next = gtp.tile([128, NPC], F16, tag="gT")
                        dense_half(relu1, w2, OUT, gT_next, 0, SPLIT)
                        stage_half(gT_next, OUT, 1, 0)
                        dense_half(relu1, w2, OUT, gT_next, SPLIT, NPC)
                        stage_half(gT_next, OUT, 1, 1)
                    else:
                        for t in range(TILES):
                            ot = outp.tile([OUT, 128], F32, tag="out")
                            nc.vector.memset(ot[:], 0.0)
                            nc.sync.dma_start(out_d[:, _ts(t, 128)], ot[:])
                    continue

                # ---- pass A: region-A chunks -> partial (+ self term) ----
                partial = prp.tile([128, NPC], F16, tag="partial")

                def finA(t, pscat, _p=partial, _g=gT, _f=Fd):
                    sl = _ts(t, 128)
                    if pscat is None:
                        nc.vector.tensor_copy(_p[:_f, sl], _g[:_f, sl])
                    else:
                        nc.vector.tensor_add(_p[:_f, sl], pscat[:_f, :],
                                             _g[:_f, sl])

                pass_region(0, Fd, range(TILES), ensure_wave, s_cache, finA)

                # ---- pass B: region-B chunks -> epilogue ----
                def finB(t, pscat, _p=partial, _f=Fd, _b=bcol, _l=layer):
                    sl = _ts(t, 128)
                    tmp2 = tmpp.tile([128, 128], F32, tag="tmp")
                    if pscat is None:
                        nc.vector.scalar_tensor_tensor(
                            tmp2[:_f, :], _p[:_f, sl], 0.0, dinv[:_f, sl],
                            mybir.AluOpType.bypass, mybir.AluOpType.mult)
                    else:
                        tmp = tmpp.tile([128, 128], F32, tag="tmp")
                        nc.vector.tensor_add(tmp[:_f, :], pscat[:_f, :],
                                             _p[:_f, sl])
                        nc.vector.scalar_tensor_tensor(
                            tmp2[:_f, :], tmp[:_f, :], 0.0, dinv[:_f, sl],
                            mybir.AluOpType.bypass, mybir.AluOpType.mult)
                    src_ap = tmp2[:_f, :]
                    if _l == 0:
                        nc.scalar.activation(
                            relu1[:_f, sl], src_ap,
                            mybir.ActivationFunctionType.Relu,
                            bias=_b[:_f, :], scale=1.0)
                    else:
                        ot = outp.tile([OUT, 128], F32, tag="out")
                        nc.scalar.activation(
                            ot[:], src_ap,
                            mybir.ActivationFunctionType.Relu,
                            bias=_b[:_f, :], scale=1.0)
                        nc.sync.dma_start(out_d[:, sl], ot[:])

                if layer == 0:
                    # tiles 0..TA-1 -> relu1 cols [0, SPLIT) -> start layer-1
                    # dense+AG for half A while pass B continues
                    pass_region(1, Fd, range(TA), ensure_wave, s_cache, finB)
                    gT_next = gtp.tile([128, NPC], F16, tag="gT")
                    dense_half(relu1, w2, OUT, gT_next, 0, SPLIT)
                    stage_half(gT_next, OUT, 1, 0)
                    pass_region(1, Fd, range(TA, TILES), ensure_wave,
                                s_cache, finB)
                    dense_half(relu1, w2, OUT, gT_next, SPLIT, NPC)
                    stage_half(gT_next, OUT, 1, 1)
                else:
                    pass_region(1, Fd, range(TILES), ensure_wave,
                                s_cache, finB)

    nc.compile()
    return nc


def _host_inputs(inputs, meta, cfg=None):
    x = np.asarray(inputs["x"], np.float32)
    W1 = np.asarray(inputs["W1"], np.float32)
    b1 = np.asarray(inputs["b1"], np.float32)
    W2 = np.asarray(inputs["W2"], np.float32)
    b2 = np.asarray(inputs["b2"], np.float32)
    dinv = meta["dinv"]

    ident = np.eye(128, dtype=NPF16)
    w1c = W1.astype(NPF16)
    w2c = W2.astype(NPF16)
    b1c = b1.reshape(HID, 1).astype(np.float32)
    b2c = b2.reshape(OUT, 1).astype(np.float32)
    iota = np.tile(np.arange(128, dtype=NPF16)[None, :], (128, 1)).copy()

    in_maps = []
    for c in range(NCORES):
        xs = np.zeros((NPC, IN), np.float32)
        xs[:NPC_REAL] = x[c * NPC_REAL:(c + 1) * NPC_REAL]
        xT = np.ascontiguousarray(xs.T).astype(NPF16)

        dv = np.ones(NPC, np.float32)
        dv[:NPC_REAL] = dinv[c * NPC_REAL:(c + 1) * NPC_REAL]
        dinvrep = np.tile(dv[None, :], (128, 1)).astype(NPF16)

        in_maps.append({
            "xT": xT, "W1": w1c, "W2": w2c, "b1c": b1c, "b2c": b2c,
            "dinvrep": dinvrep, "ident": ident,
            "idxt": meta["idx_t"][c], "dstid": meta["dstid_t"][c],
            "iota": iota,
        })
    return in_maps


def kernel(**inputs) -> np.ndarray:
    meta = _preprocess(np.asarray(inputs["edge_index"]))
    nc = _build_program(meta)
    in_maps = _host_inputs(inputs, meta)
    res = run_bass_kernel_spmd(nc, in_maps, list(range(NCORES)))
    out = np.empty((N, OUT), np.float32)
    for c in range(NCORES):
        out[c * NPC_REAL:(c + 1) * NPC_REAL] = \
            res.results[c]["outT"][:, :NPC_REAL].T
    return out


# revision 8
# speedup vs baseline: 1.8428x; 1.0222x over previous
"""Two-layer GCN (PyG GCNConv x2 + ReLU) on 8 Trainium2 NeuronCores.

Strategy (graph/data parallel, destination-partitioned):
  - Nodes row-sharded across 8 cores (6250 real + pad -> 6272 per core).
  - Edges partitioned by destination owner, grouped by destination tile
    (128 dst nodes) and by source-row region:
      region A: source local row in [0, 3200)  -> tableA rows (8*3200=25600)
      region B: source local row in [3200, 6272) -> tableB rows (8*3072=24576)
    so gather indices always fit in int16 AND the per-region AllGather can
    be overlapped with gathers of the other region.
  - Per layer on each core:  h^T = W^T @ x^T on PE;  g^T = h^T * dinv[src];
    g rows staged to DRAM in two halves, each AllGather'ed separately.
    Per-edge source rows fetched with SWDGE dma_gather (4 queues).
    Segment-sum by destination via PE matmul against a one-hot S matrix
    built ON DEVICE (iota == dstid, scaled by dinv[dst]) -- no S traffic
    from HBM.  Self-loop term g*dinv folded into the epilogue.
  - fp16 operands with fp32 PSUM accumulation.
"""

import math
import os
import sys

import numpy as np

for _p in ("/opt/trn_rl_repo", "/root/.axon_site/_ro/trn_rl_repo"):
    if os.path.isdir(_p) and _p not in sys.path:
        sys.path.append(_p)

import concourse.bacc as bacc
import concourse.bass as bass
import concourse.mybir as mybir
import concourse.tile as tile
from concourse.bass_utils import run_bass_kernel_spmd

# Problem constants (hardcoded per harness contract).
N, E, IN, HID, OUT = 50000, 800000, 128, 128, 64
NCORES = 8
NPC_REAL = N // NCORES          # 6250
TILES = 49
NPC = TILES * 128               # 6272 padded nodes per core
SPLIT = 3200                    # region A local rows (25 tiles)
NB = NPC - SPLIT                # 3072 region B local rows (24 tiles)
TA = SPLIT // 128               # 25 tiles in region A half
TB = NB // 128                  # 24
RA = NCORES * SPLIT             # 25600 tableA rows
RB = NCORES * NB                # 24576 tableB rows
WV = 16                         # gather wave size in chunks (128 slots each)

F16 = mybir.dt.float16
F32 = mybir.dt.float32
NPF16 = np.float16

_ts = bass.ts


def default_cfg():
    return dict(N=N, E=E, IN=IN, HID=HID, OUT=OUT, NCORES=NCORES,
                NPC_REAL=NPC_REAL, TILES=TILES, NPC=NPC, WV=WV)


def _preprocess(edge_index: np.ndarray, cfg=None):
    """Partition/sort/pad edges; build per-core gather-index, dst-id and
    dst-norm arrays plus the shared static chunk schedule."""
    src = np.asarray(edge_index[0], np.int64)
    dst = np.asarray(edge_index[1], np.int64)
    deg = np.bincount(dst, minlength=N).astype(np.float64) + 1.0
    dinv = 1.0 / np.sqrt(deg)

    owner = dst // NPC_REAL
    dst_local = dst % NPC_REAL
    so = src // NPC_REAL
    sl = src % NPC_REAL
    region = (sl >= SPLIT).astype(np.int64)
    row = np.where(region == 0, so * SPLIT + sl, so * NB + (sl - SPLIT))
    tile_id = dst_local // 128
    intile = dst_local % 128

    cnt = np.zeros((NCORES, TILES, 2), np.int64)
    np.add.at(cnt, (owner, tile_id, region), 1)
    K = np.ceil(cnt.max(axis=0) / 128).astype(np.int64)   # [TILES, 2]
    K_A, K_B = K[:, 0].copy(), K[:, 1].copy()
    C_A, C_B = int(K_A.sum()), int(K_B.sum())
    C = C_A + C_B
    LB = np.concatenate([[0], np.cumsum(K_A)[:-1]]).astype(np.int64)
    HB = (C_A + np.concatenate([[0], np.cumsum(K_B)[:-1]])).astype(np.int64)

    # slot assignment: sort edges by (owner, tile, region); position within
    # each (owner, tile, region) group via cumulative count
    gid = (owner * TILES + tile_id) * 2 + region
    order = np.lexsort((row, gid))
    gs = gid[order]
    starts = np.concatenate([[0], np.flatnonzero(np.diff(gs)) + 1])
    group_of = np.searchsorted(starts, np.arange(len(gs)), side="right") - 1
    pos = np.arange(len(gs)) - starts[group_of]

    base_chunk = np.where(region == 0, LB[tile_id], HB[tile_id])
    slot = np.empty(len(gs), np.int64)
    slot[order] = base_chunk[order] * 128 + pos

    nslots = C * 128
    idx16 = np.zeros((NCORES, nslots), np.int16)
    dstid = np.full((NCORES, nslots), -1.0, np.float32)
    idx16[owner, slot] = row.astype(np.int16)
    dstid[owner, slot] = intile

    # wrapped+replicated gather index tile [128, C*8]
    idx_t = idx16.reshape(NCORES, C * 8, 16).transpose(0, 2, 1)     # [8,16,C*8]
    idx_t = np.tile(idx_t, (1, 8, 1)).copy()                        # [8,128,C*8]
    dstid_t = dstid.reshape(NCORES, C, 128).transpose(0, 2, 1)      # [8,128,C]
    dstid_t = dstid_t.astype(np.float32).copy()

    return dict(deg=deg, dinv=dinv, K_A=K_A, K_B=K_B, C_A=C_A, C_B=C_B,
                C=C, LB=LB, HB=HB, idx_t=idx_t, dstid_t=dstid_t)


def _waves(n_chunks: int, wv: int = WV):
    out = []
    c = 0
    while c < n_chunks:
        n = min(wv, n_chunks - c)
        out.append((c, n))
        c += n
    return out


def _build_program(meta, cfg=None):
    stage = (cfg or {}).get("stage", os.environ.get("GCN_STAGE", "full"))
    K_A, K_B = meta["K_A"], meta["K_B"]
    C_A, C_B, C = meta["C_A"], meta["C_B"], meta["C"]
    LB, HB = meta["LB"], meta["HB"]

    nc = bacc.Bacc("TRN2", target_bir_lowering=False, debug=False,
                   num_devices=NCORES, num_swdge_queues=4)

    # ---- I/O ----
    xT_d = nc.dram_tensor("xT", [IN, NPC], F16, kind="ExternalInput")
    w1_d = nc.dram_tensor("W1", [IN, HID], F16, kind="ExternalInput")
    w2_d = nc.dram_tensor("W2", [HID, OUT], F16, kind="ExternalInput")
    b1_d = nc.dram_tensor("b1c", [HID, 1], F32, kind="ExternalInput")
    b2_d = nc.dram_tensor("b2c", [OUT, 1], F32, kind="ExternalInput")
    dinv_d = nc.dram_tensor("dinvrep", [128, NPC], F16, kind="ExternalInput")
    ident_d = nc.dram_tensor("ident", [128, 128], F16, kind="ExternalInput")
    idx_d = nc.dram_tensor("idxt", [128, C * 8], mybir.dt.int16,
                           kind="ExternalInput")
    dstid_d = nc.dram_tensor("dstid", [128, C], F32, kind="ExternalInput")
    iota_d = nc.dram_tensor("iota", [128, 128], F16, kind="ExternalInput")
    out_d = nc.dram_tensor("outT", [OUT, NPC], F32, kind="ExternalOutput")

    # ---- internal DRAM (collective bounce + replicated tables) ----
    gdA = [nc.dram_tensor(f"gdA{l}", [SPLIT, 128], F16) for l in (1, 2)]
    gdB = [nc.dram_tensor(f"gdB{l}", [NB, 128], F16) for l in (1, 2)]
    tabA = [nc.dram_tensor(f"tabA{l}", [RA, 128], F16, addr_space="Shared")
            for l in (1, 2)]
    tabB = [nc.dram_tensor(f"tabB{l}", [RB, 128], F16, addr_space="Shared")
            for l in (1, 2)]

    rg = [list(range(NCORES))]
    a_waves = _waves(C_A)
    b_waves = _waves(C_B)

    with tile.TileContext(nc) as tc:
        with (
            tc.tile_pool(name="const", bufs=1) as constp,
            tc.tile_pool(name="xtp", bufs=1) as xtp,
            tc.tile_pool(name="gtp", bufs=2) as gtp,
            tc.tile_pool(name="stp", bufs=2) as stp,
            tc.tile_pool(name="rlp", bufs=1) as rlp,
            tc.tile_pool(name="prp", bufs=2) as prp,
            tc.tile_pool(name="wap", bufs=6) as wap,
            tc.tile_pool(name="wbp", bufs=6) as wbp,
            tc.tile_pool(name="sp", bufs=32) as sp,
            tc.tile_pool(name="outp", bufs=3) as outp,
            tc.tile_pool(name="tmpp", bufs=4) as tmpp,
            tc.tile_pool(name="pmm", bufs=2, space="PSUM") as pmm,
            tc.tile_pool(name="ptr", bufs=2, space="PSUM") as ptr,
            tc.tile_pool(name="psc", bufs=4, space="PSUM") as psc,
        ):
            # ---- constants / inputs to SBUF ----
            xT = xtp.tile([128, NPC], F16, tag="xT")
            nc.sync.dma_start(xT[:IN, :], xT_d[:, :])
            dinv = constp.tile([128, NPC], F16, tag="dinv")
            nc.scalar.dma_start(dinv[:], dinv_d[:, :])
            idxt = constp.tile([128, C * 8], mybir.dt.int16, tag="idxt")
            nc.sync.dma_start(idxt[:], idx_d[:, :])
            w1 = constp.tile([IN, HID], F16, tag="w1")
            nc.scalar.dma_start(w1[:], w1_d[:, :])
            w2 = constp.tile([HID, OUT], F16, tag="w2")
            nc.scalar.dma_start(w2[:], w2_d[:, :])
            b1 = constp.tile([HID, 1], F32, tag="b1")
            nc.scalar.dma_start(b1[:], b1_d[:, :])
            b2 = constp.tile([OUT, 1], F32, tag="b2")
            nc.scalar.dma_start(b2[:], b2_d[:, :])
            ident = constp.tile([128, 128], F16, tag="ident")
            nc.scalar.dma_start(ident[:], ident_d[:, :])
            dstid = constp.tile([128, C], F32, tag="dstid")
            nc.scalar.dma_start(dstid[:], dstid_d[:, :])
            iota = constp.tile([128, 128], F16, tag="iota")
            nc.scalar.dma_start(iota[:], iota_d[:, :])

            state = dict(qn=0, s_eng=0)

            def dense_half(rhs, W, Fd, gT, c0, c1):
                """gT[:, c0:c1] = (W^T @ rhs[:, c0:c1]) * dinv  (fp16)."""
                i = c0
                while i < c1:
                    w_ = min(512, c1 - i)
                    sl = slice(i, i + w_)
                    ps = pmm.tile([128, 512], F32, tag="pmm")
                    nc.tensor.matmul(ps[:Fd, :w_], W[:, :Fd], rhs[:128, sl],
                                     start=True, stop=True)
                    nc.vector.scalar_tensor_tensor(
                        gT[:Fd, sl], ps[:Fd, :w_], 0.0, dinv[:Fd, sl],
                        mybir.AluOpType.bypass, mybir.AluOpType.mult)
                    i += w_

            def stage_half(gT, Fd, layer, half):
                """Transpose per-tile into row-major, DMA to DRAM, AllGather."""
                t0, nt = (0, TA) if half == 0 else (TA, TB)
                ncols = nt * 128
                staged = stp.tile([128, TA * 128], F16, tag="staged")
                if Fd < 128:
                    nc.vector.memset(staged[:, :ncols], 0.0)
                for t in range(t0, t0 + nt):
                    pt = ptr.tile([128, 128], F16, tag="ptr")
                    nc.tensor.transpose(pt[:, :Fd], gT[:Fd, _ts(t, 128)],
                                        ident[:Fd, :Fd])
                    dstc = (t - t0) * 128
                    nc.scalar.copy(staged[:, dstc:dstc + Fd], pt[:, :Fd])
                gd = gdA[layer] if half == 0 else gdB[layer]
                tab = tabA[layer] if half == 0 else tabB[layer]
                gview = gd.ap().rearrange("(t p) f -> p t f", p=128)
                sview = staged[:, :ncols].rearrange("p (t f) -> p t f", f=128)
                nc.sync.dma_start(gview, sview)
                nc.gpsimd.collective_compute(
                    "AllGather", mybir.AluOpType.bypass, replica_groups=rg,
                    ins=[gd.ap()], outs=[tab.ap()])

            def make_wave_cache(layer):
                cache = {}

                def ensure_wave(rgn, wi):
                    key = (rgn, wi)
                    if key in cache:
                        return cache[key]
                    c0, n = (a_waves if rgn == 0 else b_waves)[wi]
                    if rgn == 1:
                        c0 += C_A
                    pool = wap if rgn == 0 else wbp
                    g = pool.tile([128, WV, 128], F16,
                                  tag="wa" if rgn == 0 else "wb")
                    tab = tabA[layer] if rgn == 0 else tabB[layer]
                    nr = RA if rgn == 0 else RB
                    qn = state["qn"]
                    state["qn"] = (qn + 1) % 4
                    nc.gpsimd.dma_gather(
                        g[:, :n, :], tab[0:nr, :],
                        idxt[:, c0 * 8:(c0 + n) * 8],
                        n * 128, n * 128, 128, single_packet=False,
                        queue_num=qn)
                    cache[key] = g
                    return g
                return ensure_wave

            def ensure_s(ch, s_cache):
                st = s_cache.get(ch)
                if st is not None:
                    return st
                st = sp.tile([128, 128], F16, tag="s")
                nc.vector.tensor_scalar(
                    st[:, :], iota[:, :], dstid[:, ch:ch + 1],
                    scalar2=None,
                    op0=mybir.AluOpType.is_equal)
                s_cache[ch] = st
                return st

            def pass_region(rgn, Fd, t_range, ensure_wave, s_cache, finish):
                Kr = K_A if rgn == 0 else K_B
                Base = LB if rgn == 0 else HB
                off = 0 if rgn == 0 else C_A

                def prefetch(t):
                    # emit wave-gathers and S-builds for tile t ahead of the
                    # previous tile's finish, so the vector engine never
                    # head-of-line blocks PE's next accumulation chain
                    for j in range(int(Kr[t])):
                        ch = int(Base[t]) + j
                        ensure_wave(rgn, (ch - off) // WV)
                        ensure_s(ch, s_cache)

                tl = list(t_range)
                if tl:
                    prefetch(tl[0])
                for ti, t in enumerate(tl):
                    if ti + 1 < len(tl):
                        prefetch(tl[ti + 1])
                    k = int(Kr[t])
                    if k == 0:
                        finish(t, None)
                        continue
                    pscat = psc.tile([128, 128], F32, tag="psc")
                    for j in range(k):
                        ch = int(Base[t]) + j
                        rel = ch - off
                        g = ensure_wave(rgn, rel // WV)
                        pos = rel % WV
                        st = ensure_s(ch, s_cache)
                        nc.tensor.matmul(
                            pscat[:Fd, :], g[:, pos, :Fd], st[:, :],
                            start=(j == 0), stop=(j == k - 1))
                    finish(t, pscat)

            relu1 = rlp.tile([128, NPC], F16, tag="relu1")

            if stage == "ag":
                gT = gtp.tile([128, NPC], F16, tag="gT")
                dense_half(xT, w1, HID, gT, 0, SPLIT)
                stage_half(gT, HID, 0, 0)
                dense_half(xT, w1, HID, gT, SPLIT, NPC)
                stage_half(gT, HID, 0, 1)
                nc.vector.memset(relu1[:, :], 0.0)
                gT2 = gtp.tile([128, NPC], F16, tag="gT")
                dense_half(relu1, w2, OUT, gT2, 0, SPLIT)
                stage_half(gT2, OUT, 1, 0)
                dense_half(relu1, w2, OUT, gT2, SPLIT, NPC)
                stage_half(gT2, OUT, 1, 1)
                for t in range(TILES):
                    ot = outp.tile([OUT, 128], F32, tag="out")
                    nc.vector.memset(ot[:], 0.0)
                    nc.sync.dma_start(out_d[:, _ts(t, 128)], ot[:])

            gT_next = None
            for layer in ((0, 1) if stage != "ag" else ()):
                Fd = HID if layer == 0 else OUT
                bcol = b1 if layer == 0 else b2

                if layer == 0:
                    gT = gtp.tile([128, NPC], F16, tag="gT")
                    dense_half(xT, w1, Fd, gT, 0, SPLIT)
                    stage_half(gT, Fd, 0, 0)
                    dense_half(xT, w1, Fd, gT, SPLIT, NPC)
                    stage_half(gT, Fd, 0, 1)
                else:
                    gT = gT_next  # computed during layer-0 pass B

                ensure_wave = make_wave_cache(layer)
                s_cache = {}

                if stage == "gather":
                    # issue every gather, then write dummy outputs
                    for wi in range(len(a_waves)):
                        ensure_wave(0, wi)
                    for wi in range(len(b_waves)):
                        ensure_wave(1, wi)
                    if layer == 0:
                        nc.vector.memset(relu1[:, :], 0.0)
                        gT_next = gtp.tile([128, NPC], F16, tag="gT")
                        dense_half(relu1, w2, OUT, gT_next, 0, SPLIT)
                        stage_half(gT_next, OUT, 1, 0)
                        dense_half(relu1, w2, OUT, gT_next, SPLIT, NPC)
                        stage_half(gT_next, OUT, 1, 1)
                    else:
                        for t in range(TILES):
                            ot = outp.tile([OUT, 128], F32, tag="out")
                            nc.vector.memset(ot[:], 0.0)
                            nc.sync.dma_start(out_d[:, _ts(t, 128)], ot[:])
                    continue

                # ---- pass A: region-A chunks -> partial (+ self term) ----
                partial = prp.tile([128, NPC], F16, tag="partial")

                def finA(t, pscat, _p=partial, _g=gT, _f=Fd):
                    sl = _ts(t, 128)
                    if pscat is None:
                        nc.vector.tensor_copy(_p[:_f, sl], _g[:_f, sl])
                    else:
                        nc.vector.tensor_add(_p[:_f, sl], pscat[:_f, :],
                                             _g[:_f, sl])

                pass_region(0, Fd, range(TILES), ensure_wave, s_cache, finA)

                # ---- pass B: region-B chunks -> epilogue ----
                def finB(t, pscat, _p=partial, _f=Fd, _b=bcol, _l=layer):
                    sl = _ts(t, 128)
                    tmp2 = tmpp.tile([128, 128], F32, tag="tmp")
                    if pscat is None:
                        nc.vector.scalar_tensor_tensor(
                            tmp2[:_f, :], _p[:_f, sl], 0.0, dinv[:_f, sl],
                            mybir.AluOpType.bypass, mybir.AluOpType.mult)
                    else:
                        tmp = tmpp.tile([128, 128], F32, tag="tmp")
                        nc.vector.tensor_add(tmp[:_f, :], pscat[:_f, :],
                                             _p[:_f, sl])
                        nc.vector.scalar_tensor_tensor(
                            tmp2[:_f, :], tmp[:_f, :], 0.0, dinv[:_f, sl],
                            mybir.AluOpType.bypass, mybir.AluOpType.mult)
                    src_ap = tmp2[:_f, :]
                    if _l == 0:
                        nc.scalar.activation(
                            relu1[:_f, sl], src_ap,
                            mybir.ActivationFunctionType.Relu,
                            bias=_b[:_f, :], scale=1.0)
                    else:
                        ot = outp.tile([OUT, 128], F32, tag="out")
                        nc.scalar.activation(
                            ot[:], src_ap,
                            mybir.ActivationFunctionType.Relu,
                            bias=_b[:_f, :], scale=1.0)
                        nc.sync.dma_start(out_d[:, sl], ot[:])

                if layer == 0:
                    # tiles 0..TA-1 -> relu1 cols [0, SPLIT) -> start layer-1
                    # dense+AG for half A while pass B continues
                    pass_region(1, Fd, range(TA), ensure_wave, s_cache, finB)
                    gT_next = gtp.tile([128, NPC], F16, tag="gT")
                    dense_half(relu1, w2, OUT, gT_next, 0, SPLIT)
                    stage_half(gT_next, OUT, 1, 0)
                    pass_region(1, Fd, range(TA, TILES), ensure_wave,
                                s_cache, finB)
                    dense_half(relu1, w2, OUT, gT_next, SPLIT, NPC)
                    stage_half(gT_next, OUT, 1, 1)
                else:
                    pass_region(1, Fd, range(TILES), ensure_wave,
                                s_cache, finB)

    nc.compile()
    return nc


def _host_inputs(inputs, meta, cfg=None):
    x = np.asarray(inputs["x"], np.float32)
    W1 = np.asarray(inputs["W1"], np.float32)
    b1 = np.asarray(inputs["b1"], np.float32)
    W2 = np.asarray(inputs["W2"], np.float32)
    b2 = np.asarray(inputs["b2"], np.float32)
    dinv = meta["dinv"]

    ident = np.eye(128, dtype=NPF16)
    w1c = W1.astype(NPF16)
    w2c = W2.astype(NPF16)
    b1c = b1.reshape(HID, 1).astype(np.float32)
    b2c = b2.reshape(OUT, 1).astype(np.float32)
    iota = np.tile(np.arange(128, dtype=NPF16)[None, :], (128, 1)).copy()

    in_maps = []
    for c in range(NCORES):
        xs = np.zeros((NPC, IN), np.float32)
        xs[:NPC_REAL] = x[c * NPC_REAL:(c + 1) * NPC_REAL]
        xT = np.ascontiguousarray(xs.T).astype(NPF16)

        dv = np.ones(NPC, np.float32)
        dv[:NPC_REAL] = dinv[c * NPC_REAL:(c + 1) * NPC_REAL]
        dinvrep = np.tile(dv[None, :], (128, 1)).astype(NPF16)

        in_maps.append({
            "xT": xT, "W1": w1c, "W2": w2c, "b1c": b1c, "b2c": b2c,
            "dinvrep": dinvrep, "ident": ident,
            "idxt": meta["idx_t"][c], "dstid": meta["dstid_t"][c],
            "iota": iota,
        })
    return in_maps


def kernel(**inputs) -> np.ndarray:
    meta = _preprocess(np.asarray(inputs["edge_index"]))
    nc = _build_program(meta)
    in_maps = _host_inputs(inputs, meta)
    res = run_bass_kernel_spmd(nc, in_maps, list(range(NCORES)))
    out = np.empty((N, OUT), np.float32)
    for c in range(NCORES):
        out[c * NPC_REAL:(c + 1) * NPC_REAL] = \
            res.results[c]["outT"][:, :NPC_REAL].T
    return out


# revision 10
# speedup vs baseline: 2.2749x; 1.2345x over previous
"""Two-layer GCN (PyG GCNConv x2 + ReLU) on 8 Trainium2 NeuronCores.

Strategy (graph/data parallel, destination-partitioned):
  - Nodes row-sharded across 8 cores (6250 real + pad -> 6272 per core).
  - Edges partitioned by destination owner, grouped by destination tile
    (128 dst nodes) and by source-row region:
      region A: source local row in [0, 3200)  -> tableA rows (8*3200=25600)
      region B: source local row in [3200, 6272) -> tableB rows (8*3072=24576)
    so gather indices always fit in int16 AND the per-region AllGather can
    be overlapped with gathers of the other region.
  - Per layer on each core: h^T = W^T @ x^T on PE (tables store RAW h);
    h rows staged to DRAM in two halves, each AllGather'ed separately.
    Per-edge source rows fetched with SWDGE dma_gather (4 queues).
    Segment-sum by destination via PE matmul against host-built S matrices
    whose values carry the full edge norm dinv[src]*dinv[dst]; the
    self-loop term is one extra PE matmul per tile with a host diag
    dinv^2 operand.  Epilogue is a pure Activation-engine Relu+bias.
  - The steady-state loop deliberately runs NOTHING on the vector (DVE)
    engine: concurrent DVE activity throttles SWDGE descriptor
    generation on the Pool DSP (measured 2ns/desc idle -> 8ns/desc busy),
    which is the throughput floor of this kernel.
  - fp16 operands with fp32 PSUM accumulation.
"""

import math
import os
import sys

import numpy as np

for _p in ("/opt/trn_rl_repo", "/root/.axon_site/_ro/trn_rl_repo"):
    if os.path.isdir(_p) and _p not in sys.path:
        sys.path.append(_p)

import concourse.bacc as bacc
import concourse.bass as bass
import concourse.mybir as mybir
import concourse.tile as tile
from concourse.bass_utils import run_bass_kernel_spmd

# Problem constants (hardcoded per harness contract).
N, E, IN, HID, OUT = 50000, 800000, 128, 128, 64
NCORES = 8
NPC_REAL = N // NCORES          # 6250
TILES = 49
NPC = TILES * 128               # 6272 padded nodes per core
SPLIT = 3200                    # region A local rows (25 tiles)
NB = NPC - SPLIT                # 3072 region B local rows (24 tiles)
TA = SPLIT // 128               # 25 tiles in region A half
TB = NB // 128                  # 24
RA = NCORES * SPLIT             # 25600 tableA rows
RB = NCORES * NB                # 24576 tableB rows
WV = 16                         # gather wave size in chunks (128 slots each)

F16 = mybir.dt.float16
F32 = mybir.dt.float32
NPF16 = np.float16

_ts = bass.ts


def default_cfg():
    return dict(N=N, E=E, IN=IN, HID=HID, OUT=OUT, NCORES=NCORES,
                NPC_REAL=NPC_REAL, TILES=TILES, NPC=NPC, WV=WV)


def _preprocess(edge_index: np.ndarray, cfg=None):
    """Partition/sort/pad edges; build per-core gather-index and S-matrix
    arrays plus the shared static chunk schedule."""
    src = np.asarray(edge_index[0], np.int64)
    dst = np.asarray(edge_index[1], np.int64)
    deg = np.bincount(dst, minlength=N).astype(np.float64) + 1.0
    dinv = 1.0 / np.sqrt(deg)

    owner = dst // NPC_REAL
    dst_local = dst % NPC_REAL
    so = src // NPC_REAL
    sl = src % NPC_REAL
    region = (sl >= SPLIT).astype(np.int64)
    row = np.where(region == 0, so * SPLIT + sl, so * NB + (sl - SPLIT))
    tile_id = dst_local // 128
    intile = dst_local % 128

    cnt = np.zeros((NCORES, TILES, 2), np.int64)
    np.add.at(cnt, (owner, tile_id, region), 1)
    K = np.ceil(cnt.max(axis=0) / 128).astype(np.int64)   # [TILES, 2]
    K_A, K_B = K[:, 0].copy(), K[:, 1].copy()
    C_A, C_B = int(K_A.sum()), int(K_B.sum())
    C = C_A + C_B
    LB = np.concatenate([[0], np.cumsum(K_A)[:-1]]).astype(np.int64)
    HB = (C_A + np.concatenate([[0], np.cumsum(K_B)[:-1]])).astype(np.int64)

    # slot assignment: sort edges by (owner, tile, region); position within
    # each (owner, tile, region) group via cumulative count
    gid = (owner * TILES + tile_id) * 2 + region
    order = np.lexsort((row, gid))
    gs = gid[order]
    starts = np.concatenate([[0], np.flatnonzero(np.diff(gs)) + 1])
    group_of = np.searchsorted(starts, np.arange(len(gs)), side="right") - 1
    pos = np.arange(len(gs)) - starts[group_of]

    base_chunk = np.where(region == 0, LB[tile_id], HB[tile_id])
    slot = np.empty(len(gs), np.int64)
    slot[order] = base_chunk[order] * 128 + pos

    nslots = C * 128
    idx16 = np.zeros((NCORES, nslots), np.int16)
    idx16[owner, slot] = row.astype(np.int16)

    # host-built S: [NCORES, 128(slot-in-chunk), C*128(chunk,dstcol)], with
    # the full symmetric norm folded into the one-hot values
    norm = (dinv[src] * dinv[dst]).astype(NPF16)
    sall = np.zeros((NCORES, nslots, 128), NPF16)
    sall[owner, slot, intile] = norm
    sall = sall.reshape(NCORES, C, 128, 128).transpose(0, 2, 1, 3) \
               .reshape(NCORES, 128, C * 128).copy()

    # self-loop diag operand: [NCORES, 128, NPC], block t = diag(dinv^2)
    sself = np.zeros((NCORES, 128, NPC), NPF16)
    nodes = np.arange(NPC_REAL, dtype=np.int64)
    for c in range(NCORES):
        d2 = (dinv[c * NPC_REAL:(c + 1) * NPC_REAL] ** 2).astype(NPF16)
        sself[c, nodes % 128, nodes] = d2

    # wrapped+replicated gather index tile [128, C*8]
    idx_t = idx16.reshape(NCORES, C * 8, 16).transpose(0, 2, 1)     # [8,16,C*8]
    idx_t = np.tile(idx_t, (1, 8, 1)).copy()                        # [8,128,C*8]

    return dict(deg=deg, dinv=dinv, K_A=K_A, K_B=K_B, C_A=C_A, C_B=C_B,
                C=C, LB=LB, HB=HB, idx_t=idx_t, sall=sall, sself=sself)


def _waves(n_chunks: int, wv: int = WV):
    out = []
    c = 0
    while c < n_chunks:
        n = min(wv, n_chunks - c)
        out.append((c, n))
        c += n
    return out


def _build_program(meta, cfg=None):
    stage = (cfg or {}).get("stage", os.environ.get("GCN_STAGE", "full"))
    K_A, K_B = meta["K_A"], meta["K_B"]
    C_A, C_B, C = meta["C_A"], meta["C_B"], meta["C"]
    LB, HB = meta["LB"], meta["HB"]

    nc = bacc.Bacc("TRN2", target_bir_lowering=False, debug=False,
                   num_devices=NCORES, num_swdge_queues=4)

    # ---- I/O ----
    xT_d = nc.dram_tensor("xT", [IN, NPC], F16, kind="ExternalInput")
    w1_d = nc.dram_tensor("W1", [IN, HID], F16, kind="ExternalInput")
    w2_d = nc.dram_tensor("W2", [HID, OUT], F16, kind="ExternalInput")
    b1_d = nc.dram_tensor("b1c", [HID, 1], F32, kind="ExternalInput")
    b2_d = nc.dram_tensor("b2c", [OUT, 1], F32, kind="ExternalInput")
    ident_d = nc.dram_tensor("ident", [128, 128], F16, kind="ExternalInput")
    idx_d = nc.dram_tensor("idxt", [128, C * 8], mybir.dt.int16,
                           kind="ExternalInput")
    s_d = nc.dram_tensor("sall", [128, C * 128], F16, kind="ExternalInput")
    sself_d = nc.dram_tensor("sself", [128, NPC], F16, kind="ExternalInput")
    out_d = nc.dram_tensor("outT", [OUT, NPC], F32, kind="ExternalOutput")

    # ---- internal DRAM (collective bounce + replicated tables) ----
    gdA = [nc.dram_tensor(f"gdA{l}", [SPLIT, 128], F16) for l in (1, 2)]
    gdB = [nc.dram_tensor(f"gdB{l}", [NB, 128], F16) for l in (1, 2)]
    tabA = [nc.dram_tensor(f"tabA{l}", [RA, 128], F16, addr_space="Shared")
            for l in (1, 2)]
    tabB = [nc.dram_tensor(f"tabB{l}", [RB, 128], F16, addr_space="Shared")
            for l in (1, 2)]

    rg = [list(range(NCORES))]
    a_waves = _waves(C_A)
    b_waves = _waves(C_B)

    with tile.TileContext(nc) as tc:
        with (
            tc.tile_pool(name="const", bufs=1) as constp,
            tc.tile_pool(name="xtp", bufs=1) as xtp,
            tc.tile_pool(name="gtp", bufs=2) as gtp,
            tc.tile_pool(name="stgp", bufs=2) as stgp,
            tc.tile_pool(name="rlp", bufs=1) as rlp,
            tc.tile_pool(name="prp", bufs=2) as prp,
            tc.tile_pool(name="wap", bufs=6) as wap,
            tc.tile_pool(name="wbp", bufs=6) as wbp,
            tc.tile_pool(name="sp", bufs=6) as sp,
            tc.tile_pool(name="outp", bufs=3) as outp,
            tc.tile_pool(name="pmm", bufs=2, space="PSUM") as pmm,
            tc.tile_pool(name="ptr", bufs=2, space="PSUM") as ptr,
            tc.tile_pool(name="psc", bufs=4, space="PSUM") as psc,
        ):
            # ---- constants / inputs to SBUF ----
            xT = xtp.tile([128, NPC], F16, tag="xT")
            nc.sync.dma_start(xT[:IN, :], xT_d[:, :])
            idxt = constp.tile([128, C * 8], mybir.dt.int16, tag="idxt")
            nc.sync.dma_start(idxt[:], idx_d[:, :])
            sself = constp.tile([128, NPC], F16, tag="sself")
            nc.sync.dma_start(sself[:], sself_d[:, :])
            w1 = constp.tile([IN, HID], F16, tag="w1")
            nc.scalar.dma_start(w1[:], w1_d[:, :])
            w2 = constp.tile([HID, OUT], F16, tag="w2")
            nc.scalar.dma_start(w2[:], w2_d[:, :])
            b1 = constp.tile([HID, 1], F32, tag="b1")
            nc.scalar.dma_start(b1[:], b1_d[:, :])
            b2 = constp.tile([OUT, 1], F32, tag="b2")
            nc.scalar.dma_start(b2[:], b2_d[:, :])
            ident = constp.tile([128, 128], F16, tag="ident")
            nc.scalar.dma_start(ident[:], ident_d[:, :])

            state = dict(qn=0)

            def dense_half(rhs, W, Fd, hT, c0, c1):
                """hT[:, c0:c1] = W^T @ rhs[:, c0:c1]  (raw h, fp16)."""
                i = c0
                while i < c1:
                    w_ = min(512, c1 - i)
                    sl = slice(i, i + w_)
                    ps = pmm.tile([128, 512], F32, tag="pmm")
                    nc.tensor.matmul(ps[:Fd, :w_], W[:, :Fd], rhs[:128, sl],
                                     start=True, stop=True)
                    nc.scalar.copy(hT[:Fd, sl], ps[:Fd, :w_])
                    i += w_

            def transpose_half(hT, Fd, staged, half):
                """Transpose per-tile into node-major staged rows."""
                t0, nt = (0, TA) if half == 0 else (TA, TB)
                for t in range(t0, t0 + nt):
                    pt = ptr.tile([128, 128], F16, tag="ptr")
                    nc.tensor.transpose(pt[:, :Fd], hT[:Fd, _ts(t, 128)],
                                        ident[:Fd, :Fd])
                    nc.scalar.copy(staged[:, t * 128:t * 128 + Fd],
                                   pt[:, :Fd])

            def stage_half(staged, layer, half):
                """staged rows -> DRAM; AllGather into the shared table."""
                gd = gdA[layer] if half == 0 else gdB[layer]
                tab = tabA[layer] if half == 0 else tabB[layer]
                lo = 0 if half == 0 else SPLIT
                ncols = SPLIT if half == 0 else NB
                gview = gd.ap().rearrange("(t p) f -> p t f", p=128)
                sview = staged[:, lo:lo + ncols].rearrange(
                    "p (t f) -> p t f", f=128)
                nc.sync.dma_start(gview, sview)
                nc.gpsimd.collective_compute(
                    "AllGather", mybir.AluOpType.bypass, replica_groups=rg,
                    ins=[gd.ap()], outs=[tab.ap()])

            def make_caches(layer):
                wcache, scache = {}, {}

                def ensure_wave(rgn, wi):
                    key = (rgn, wi)
                    if key in wcache:
                        return wcache[key]
                    c0, n = (a_waves if rgn == 0 else b_waves)[wi]
                    if rgn == 1:
                        c0 += C_A
                    pool = wap if rgn == 0 else wbp
                    g = pool.tile([128, WV, 128], F16,
                                  tag="wa" if rgn == 0 else "wb")
                    tab = tabA[layer] if rgn == 0 else tabB[layer]
                    nr = RA if rgn == 0 else RB
                    qn = state["qn"]
                    state["qn"] = (qn + 1) % 4
                    nc.gpsimd.dma_gather(
                        g[:, :n, :], tab[0:nr, :],
                        idxt[:, c0 * 8:(c0 + n) * 8],
                        n * 128, n * 128, 128, single_packet=False,
                        queue_num=qn)
                    wcache[key] = g
                    return g

                def ensure_s(rgn, wi):
                    key = (rgn, wi)
                    if key in scache:
                        return scache[key]
                    c0, n = (a_waves if rgn == 0 else b_waves)[wi]
                    if rgn == 1:
                        c0 += C_A
                    st = sp.tile([128, WV * 128], F16, tag="s")
                    nc.scalar.dma_start(st[:, :n * 128],
                                        s_d[:, c0 * 128:(c0 + n) * 128])
                    scache[key] = st
                    return st

                return ensure_wave, ensure_s

            def pass_region(rgn, Fd, t_range, ensure_wave, ensure_s,
                            first_mm, finish):
                """first_mm(t, pscat, is_last) emits the leading PSUM matmul
                (self chunk for pass A, partial-inject for pass B); regular
                chunks accumulate on top; finish(t, pscat) drains."""
                Kr = K_A if rgn == 0 else K_B
                Base = LB if rgn == 0 else HB
                off = 0 if rgn == 0 else C_A

                def prefetch(t):
                    for j in range(int(Kr[t])):
                        ch = int(Base[t]) + j
                        rel = ch - off
                        ensure_wave(rgn, rel // WV)
                        ensure_s(rgn, rel // WV)

                tl = list(t_range)
                if tl:
                    prefetch(tl[0])
                for ti, t in enumerate(tl):
                    if ti + 1 < len(tl):
                        prefetch(tl[ti + 1])
                    k = int(Kr[t])
                    pscat = psc.tile([128, 128], F32, tag="psc")
                    first_mm(t, pscat, k == 0)
                    for j in range(k):
                        ch = int(Base[t]) + j
                        rel = ch - off
                        g = ensure_wave(rgn, rel // WV)
                        pos = rel % WV
                        sw = ensure_s(rgn, rel // WV)
                        spos = rel % WV
                        nc.tensor.matmul(
                            pscat[:Fd, :], g[:, pos, :Fd],
                            sw[:, spos * 128:(spos + 1) * 128],
                            start=False, stop=(j == k - 1))
                    finish(t, pscat)

            relu1 = rlp.tile([128, NPC], F16, tag="relu1")

            gT_next = None
            staged_next = None
            for layer in (0, 1):
                Fd = HID if layer == 0 else OUT
                bcol = b1 if layer == 0 else b2

                if layer == 0:
                    hT = gtp.tile([128, NPC], F16, tag="hT")
                    staged = stgp.tile([128, NPC], F16, tag="staged")
                    dense_half(xT, w1, Fd, hT, 0, SPLIT)
                    transpose_half(hT, Fd, staged, 0)
                    stage_half(staged, 0, 0)
                    dense_half(xT, w1, Fd, hT, SPLIT, NPC)
                    transpose_half(hT, Fd, staged, 1)
                    stage_half(staged, 0, 1)
                else:
                    hT = gT_next
                    staged = staged_next

                ensure_wave, ensure_s = make_caches(layer)

                if stage == "gather":
                    for wi in range(len(a_waves)):
                        ensure_wave(0, wi)
                    for wi in range(len(b_waves)):
                        ensure_wave(1, wi)
                    if layer == 0:
                        nc.vector.memset(relu1[:, :], 0.0)
                        gT_next = gtp.tile([128, NPC], F16, tag="hT")
                        staged_next = stgp.tile([128, NPC], F16,
                                                tag="staged")
                        nc.vector.memset(staged_next[:, :], 0.0)
                        dense_half(relu1, w2, OUT, gT_next, 0, SPLIT)
                        transpose_half(gT_next, OUT, staged_next, 0)
                        stage_half(staged_next, 1, 0)
                        dense_half(relu1, w2, OUT, gT_next, SPLIT, NPC)
                        transpose_half(gT_next, OUT, staged_next, 1)
                        stage_half(staged_next, 1, 1)
                    else:
                        for t in range(TILES):
                            ot = outp.tile([OUT, 128], F32, tag="out")
                            nc.vector.memset(ot[:], 0.0)
                            nc.sync.dma_start(out_d[:, _ts(t, 128)], ot[:])
                    continue

                # ---- pass A: self-loop chunk + region-A chunks ----
                partial = prp.tile([128, NPC], F16, tag="partial")

                def firstA(t, pscat, is_last, _st=staged, _f=Fd):
                    sl = _ts(t, 128)
                    nc.tensor.matmul(
                        pscat[:_f, :], _st[:, t * 128:t * 128 + _f],
                        sself[:, sl], start=True, stop=is_last)

                def finA(t, pscat, _p=partial, _f=Fd):
                    nc.scalar.copy(_p[:_f, _ts(t, 128)], pscat[:_f, :])

                pass_region(0, Fd, range(TILES), ensure_wave, ensure_s,
                            firstA, finA)

                # ---- pass B: partial-inject chunk + region-B chunks ----
                def firstB(t, pscat, is_last, _p=partial, _f=Fd):
                    nc.tensor.matmul(
                        pscat[:_f, :], ident[:_f, :_f],
                        _p[:_f, _ts(t, 128)], start=True, stop=is_last)

                def finB(t, pscat, _f=Fd, _b=bcol, _l=layer):
                    sl = _ts(t, 128)
                    if _l == 0:
                        nc.scalar.activation(
                            relu1[:_f, sl], pscat[:_f, :],
                            mybir.ActivationFunctionType.Relu,
                            bias=_b[:_f, :], scale=1.0)
                    else:
                        ot = outp.tile([OUT, 128], F32, tag="out")
                        nc.scalar.activation(
                            ot[:], pscat[:_f, :],
                            mybir.ActivationFunctionType.Relu,
                            bias=_b[:_f, :], scale=1.0)
                        nc.sync.dma_start(out_d[:, sl], ot[:])

                if layer == 0:
                    # tiles 0..TA-1 -> relu1 cols [0, SPLIT) -> start layer-1
                    # dense+AG for half A while pass B continues
                    pass_region(1, Fd, range(TA), ensure_wave, ensure_s,
                                firstB, finB)
                    gT_next = gtp.tile([128, NPC], F16, tag="hT")
                    staged_next = stgp.tile([128, NPC], F16, tag="staged")
                    nc.vector.memset(staged_next[:, :], 0.0)
                    dense_half(relu1, w2, OUT, gT_next, 0, SPLIT)
                    transpose_half(gT_next, OUT, staged_next, 0)
                    stage_half(staged_next, 1, 0)
                    pass_region(1, Fd, range(TA, TILES), ensure_wave,
                                ensure_s, firstB, finB)
                    dense_half(relu1, w2, OUT, gT_next, SPLIT, NPC)
                    transpose_half(gT_next, OUT, staged_next, 1)
                    stage_half(staged_next, 1, 1)
                else:
                    pass_region(1, Fd, range(TILES), ensure_wave,
                                ensure_s, firstB, finB)

    nc.compile()
    return nc


def _host_inputs(inputs, meta, cfg=None):
    x = np.asarray(inputs["x"], np.float32)
    W1 = np.asarray(inputs["W1"], np.float32)
    b1 = np.asarray(inputs["b1"], np.float32)
    W2 = np.asarray(inputs["W2"], np.float32)
    b2 = np.asarray(inputs["b2"], np.float32)

    ident = np.eye(128, dtype=NPF16)
    w1c = W1.astype(NPF16)
    w2c = W2.astype(NPF16)
    b1c = b1.reshape(HID, 1).astype(np.float32)
    b2c = b2.reshape(OUT, 1).astype(np.float32)

    in_maps = []
    for c in range(NCORES):
        xs = np.zeros((NPC, IN), np.float32)
        xs[:NPC_REAL] = x[c * NPC_REAL:(c + 1) * NPC_REAL]
        xT = np.ascontiguousarray(xs.T).astype(NPF16)

        in_maps.append({
            "xT": xT, "W1": w1c, "W2": w2c, "b1c": b1c, "b2c": b2c,
            "ident": ident, "idxt": meta["idx_t"][c],
            "sall": meta["sall"][c], "sself": meta["sself"][c],
        })
    return in_maps


def kernel(**inputs) -> np.ndarray:
    meta = _preprocess(np.asarray(inputs["edge_index"]))
    nc = _build_program(meta)
    in_maps = _host_inputs(inputs, meta)
    res = run_bass_kernel_spmd(nc, in_maps, list(range(NCORES)))
    out = np.empty((N, OUT), np.float32)
    for c in range(NCORES):
        out[c * NPC_REAL:(c + 1) * NPC_REAL] = \
            res.results[c]["outT"][:, :NPC_REAL].T
    return out
